# revision 12
# baseline (speedup 1.0000x reference)
"""Trainium2 Bass kernel for nn_DilateMask (16x1x1024x1024 f32 mask, 5 iterations).

The reference iterates 5x: edge-pad, 3x3 discrete-Laplacian conv, then
`mask[|res| > 1e-4] = 1`. For a binary mask this is exactly binary dilation by
the L1 diamond of radius 5 (edge padding replicates the center pixel, which is
0 in the only case that matters).

Decomposition (pure data parallel, 2 images per core x 8 cores; 118-row output
chunks with 5-row halos so chunks are independent):
  1. HWDGE (sync) DMA loads mask rows into an f32 tile; one DVE tensor_copy
     casts to bf16 (single-src copy runs at 2x_2p; SWDGE cast-DMA measured
     ~46us slower end to end).
  2. Horizontal decay field on DVE in 3 ops, all at 2x mode (B = 1/128):
       a[c]  = max(v0[c-4], v0[c+2])          (aligned tensor_tensor, 2x_1p)
       ts[j] = B^3 * a[j+1]                   (tensor_scalar; the odd shift
                                               rides on the single-src op,
                                               which hits 2x_2p regardless
                                               of alignment)
       v1[j] = max(ts[j], v0[j])              (aligned tensor_tensor, 2x_1p)
     giving v1(j) = max(m(j), B^3 m(j-3), B^3 m(j+3)).
  3. Five column-shifted matmuls per 512-block accumulate into PSUM with
     Toeplitz band weights:
       T(i,j) = sum_{|dj|<=2} sum_{|di|<=5} B^(|dj|+|di|) v1(i+di, j+dj)
     The exact-decay paths give any set pixel at L1 distance d<=5 a term
     >= B^5; pixels at distance >=6 contribute <= 55*B^6 = 0.43*B^5 total.
     (All weights/values are exact powers of two in bf16; fp32 accumulation
     of positive terms never drops below the largest term.)
  4. Threshold on ACT in two full-width ops: sign(T - 0.7*B^5) in {-1,+1},
     then 0.5*x + 0.5 -> exactly {0.0, 1.0} f32. (Sigmoid would do it in one
     op but pays a ~2.7us activation-table load per call on this toolchain.)
  5. Output DMA via SWDGE (gpsimd) so output waits never block input issue.

Built on bacc.Bacc: this toolchain's instruction encodings hold only one
sync-wait each, and Bacc legalizes Tile's multi-wait instructions.
"""
import numpy as np
import ml_dtypes
from contextlib import ExitStack

import concourse.bacc as bacc
import concourse.mybir as mybir
import concourse.tile as tile

B, C, H, W = 16, 1, 1024, 1024
N_CORES = 8
PER_CORE = B // N_CORES
CO = 118               # output rows per chunk (118 + 2*5 halo = 128 input rows)
R = 5                  # dilation radius
BETA = 2.0 ** -7
B3 = BETA ** 3
TAU = 0.7 * BETA ** R
PAD = 8                # zero-pad columns each side of the 1024-wide interior
WP = W + 2 * PAD
NBLK = 512             # PSUM bank width in f32
NSLOT = 8              # software pipeline depth (SBUF tiles)
NSLOT_PS = 4           # PSUM pipeline depth (2 banks per slot, 8 banks total)


def _band_stack():
    """bands[v, s, k, m] = BETA**(s + |di|), di = k - m - (0 if v==0 else R).

    v=0: top chunk (no upper halo), v=1: interior/bottom chunks.
    s in {0,1,2} is the |dj| scale baked into the stationary operand.
    """
    out = np.zeros((2, 3, 128, CO), dtype=np.float64)
    k = np.arange(128)[:, None]
    m = np.arange(CO)[None, :]
    for v, off in ((0, 0), (1, R)):
        d = np.abs(k - m - off)
        base = np.where(d <= R, float(BETA) ** d, 0.0)
        for s in range(3):
            out[v, s] = base * (float(BETA) ** s)
    return np.ascontiguousarray(out.astype(ml_dtypes.bfloat16))


def _const_inputs():
    return {"bands": _band_stack()}


def _build(nc, n_img, loop_niter=None):
    """loop_niter: wrap the body in a device-side For_i (timing only)."""
    x = nc.dram_tensor("x", [n_img, H, W], mybir.dt.float32, kind="ExternalInput")
    bands = nc.dram_tensor("bands", [2, 3, 128, CO], mybir.dt.bfloat16,
                           kind="ExternalInput")
    y = nc.dram_tensor("y", [n_img, H, W], mybir.dt.float32, kind="ExternalOutput")
    n_chunks = (H + CO - 1) // CO

    with tile.TileContext(nc) as tc, ExitStack() as ctx:
        wpool = ctx.enter_context(tc.tile_pool(name="weights", bufs=1))
        vpool = ctx.enter_context(tc.tile_pool(name="v", bufs=1))
        opool = ctx.enter_context(tc.tile_pool(name="out", bufs=1))
        ppool = ctx.enter_context(tc.tile_pool(name="psum", bufs=1, space="PSUM"))

        band_t = [[wpool.tile([128, CO], mybir.dt.bfloat16, name=f"band_{v}_{s}")
                   for s in range(3)] for v in range(2)]
        for v in range(2):
            for s in range(3):
                nc.sync.dma_start(band_t[v][s][:], bands[v, s])

        v0s = [vpool.tile([128, WP], mybir.dt.bfloat16, name=f"v0_{i}")
               for i in range(NSLOT)]
        ats = [vpool.tile([128, WP], mybir.dt.bfloat16, name=f"a_{i}")
               for i in range(NSLOT)]
        tss = [vpool.tile([128, W], mybir.dt.bfloat16, name=f"ts_{i}")
               for i in range(NSLOT)]
        v1s = [vpool.tile([128, WP], mybir.dt.bfloat16, name=f"v1_{i}")
               for i in range(NSLOT)]
        ots = [opool.tile([128, W], mybir.dt.float32, name=f"ot_{i}")
               for i in range(NSLOT)]
        pss = [ppool.tile([128, 2 * NBLK], mybir.dt.float32, name=f"ps_{i}")
               for i in range(NSLOT_PS)]
        mts = [vpool.tile([128, W], mybir.dt.float32, name=f"mt_{i}")
               for i in range(NSLOT)]
        sgs = [vpool.tile([128, W], mybir.dt.bfloat16, name=f"sg_{i}")
               for i in range(NSLOT)]
        ntau_t = wpool.tile([128, 1], mybir.dt.float32, name="ntau_t")
        nc.vector.memset(ntau_t[:], -float(TAU))
        half_t = wpool.tile([128, 1], mybir.dt.float32, name="half_t")
        nc.vector.memset(half_t[:], 0.5)
        for t in (*v0s, *v1s):
            nc.vector.memset(t[:, :PAD], 0.0)
            nc.vector.memset(t[:, PAD + W:], 0.0)

        loop = tc.For_i(0, loop_niter, 1) if loop_niter else None
        if loop:
            loop.__enter__()
        cidx = 0
        for img in range(n_img):
            for c in range(n_chunks):
                o0 = c * CO
                o1 = min(o0 + CO, H)
                i0 = max(o0 - R, 0)
                i1 = min(o1 + R, H)
                K = i1 - i0
                M = o1 - o0
                s = cidx % NSLOT
                ps = pss[cidx % NSLOT_PS]
                cidx += 1
                v0, a, ts, v1, ot = v0s[s], ats[s], tss[s], v1s[s], ots[s]
                mt, sg = mts[s], sgs[s]

                in_eng = nc.sync if (cidx % 2) else nc.scalar
                in_eng.dma_start(mt[:K, :], x[img, i0:i1, :])
                nc.vector.tensor_copy(v0[:K, PAD:PAD + W], mt[:K, :])

                nc.vector.tensor_tensor(
                    out=a[:K, PAD:PAD + W + 2],
                    in0=v0[:K, PAD - 4:PAD - 4 + W + 2],
                    in1=v0[:K, PAD + 2:PAD + 2 + W + 2],
                    op=mybir.AluOpType.max)
                nc.vector.tensor_scalar(
                    out=ts[:K, :], in0=a[:K, PAD + 1:PAD + 1 + W],
                    scalar1=float(B3), scalar2=None,
                    op0=mybir.AluOpType.mult)
                nc.vector.tensor_tensor(
                    out=v1[:K, PAD:PAD + W],
                    in0=ts[:K, :], in1=v0[:K, PAD:PAD + W],
                    op=mybir.AluOpType.max)

                bv = band_t[0 if c == 0 else 1]
                nmm = [0, 0]
                for s_idx, djs in ((2, (-2, 2)), (1, (-1, 1)), (0, (0,))):
                    lhsT = bv[s_idx][:K, :M]
                    for dj in djs:
                        for h in (0, 1):
                            col = PAD + h * NBLK + dj
                            nc.tensor.matmul(
                                ps[:M, h * NBLK:(h + 1) * NBLK],
                                lhsT, v1[:K, col:col + NBLK],
                                start=(nmm[h] == 0), stop=(nmm[h] == 4))
                            nmm[h] += 1

                nc.scalar.activation(sg[:M, :], ps[:M, :],
                                     mybir.ActivationFunctionType.Sign,
                                     bias=ntau_t[:M])
                nc.scalar.activation(ot[:M, :], sg[:M, :],
                                     mybir.ActivationFunctionType.Identity,
                                     bias=half_t[:M], scale=0.5)
                nc.gpsimd.dma_start(y[img, o0:o1, :], ot[:M, :])
        if loop:
            loop.__exit__(None, None, None)
    return nc


_CACHE = {}


def _get_nc():
    if "nc" not in _CACHE:
        nc = bacc.Bacc("TRN2", target_bir_lowering=False)
        _build(nc, PER_CORE)
        nc.compile()
        _CACHE["nc"] = nc
        _CACHE["consts"] = _const_inputs()
    return _CACHE["nc"], _CACHE["consts"]


def kernel(batch_mask, weight=None, iter_num=None, **_unused):
    from concourse.bass_utils import run_bass_kernel_spmd

    nc, consts = _get_nc()
    bm4 = np.ascontiguousarray(np.asarray(batch_mask, dtype=np.float32))
    assert bm4.shape == (B, C, H, W), bm4.shape
    in_maps = []
    for cidx in range(N_CORES):
        xs = np.ascontiguousarray(bm4[cidx * PER_CORE:(cidx + 1) * PER_CORE, 0])
        in_maps.append({"x": xs, **consts})
    res = run_bass_kernel_spmd(nc, in_maps, list(range(N_CORES)))
    out = np.concatenate([np.asarray(res.results[cidx]["y"])
                          for cidx in range(N_CORES)], axis=0)
    return out.reshape(B, C, H, W).astype(np.float32)


# revision 13
# speedup vs baseline: 1.7995x; 1.7995x over previous
"""Trainium2 Bass kernel for nn_DilateMask (16x1x1024x1024 f32 mask, 5 iterations).

The reference iterates 5x: edge-pad, 3x3 discrete-Laplacian conv, then
`mask[|res| > 1e-4] = 1`. For a binary mask this is exactly binary dilation by
the L1 diamond of radius 5 (edge padding replicates the center pixel, which is
0 in the only case that matters).

Decomposition (pure data parallel, 2 images per core x 8 cores; 118-row output
chunks with 5-row halos so chunks are independent):
  1. HWDGE (sync) DMA loads mask rows into an f32 tile; one DVE tensor_copy
     casts to bf16 (single-src copy runs at 2x_2p; SWDGE cast-DMA measured
     ~46us slower end to end).
  2. Horizontal decay field on DVE in 3 ops, all at 2x mode (B = 1/128):
       a[c]  = max(v0[c-4], v0[c+2])          (aligned tensor_tensor, 2x_1p)
       ts[j] = B^3 * a[j+1]                   (tensor_scalar; the odd shift
                                               rides on the single-src op,
                                               which hits 2x_2p regardless
                                               of alignment)
       v1[j] = max(ts[j], v0[j])              (aligned tensor_tensor, 2x_1p)
     giving v1(j) = max(m(j), B^3 m(j-3), B^3 m(j+3)).
  3. Five column-shifted matmuls per 512-block accumulate into PSUM with
     Toeplitz band weights:
       T(i,j) = sum_{|dj|<=2} sum_{|di|<=5} B^(|dj|+|di|) v1(i+di, j+dj)
     The exact-decay paths give any set pixel at L1 distance d<=5 a term
     >= B^5; pixels at distance >=6 contribute <= 55*B^6 = 0.43*B^5 total.
     (All weights/values are exact powers of two in bf16; fp32 accumulation
     of positive terms never drops below the largest term.)
  4. Threshold on ACT in two full-width ops: sign(T - 0.7*B^5) in {-1,+1},
     then 0.5*x + 0.5 -> exactly {0.0, 1.0} f32. (Sigmoid would do it in one
     op but pays a ~2.7us activation-table load per call on this toolchain.)
  5. Output DMA via SWDGE (gpsimd) so output waits never block input issue.

Built on bacc.Bacc: this toolchain's instruction encodings hold only one
sync-wait each, and Bacc legalizes Tile's multi-wait instructions.
"""
import numpy as np
import ml_dtypes
from contextlib import ExitStack

import concourse.bacc as bacc
import concourse.mybir as mybir
import concourse.tile as tile

B, C, H, W = 16, 1, 1024, 1024
N_CORES = 8
PER_CORE = B // N_CORES
CO = 118               # output rows per chunk (118 + 2*5 halo = 128 input rows)
R = 5                  # dilation radius
BETA = 2.0 ** -7
B3 = BETA ** 3
TAU = 0.7 * BETA ** R
PAD = 8                # zero-pad columns each side of the 1024-wide interior
WP = W + 2 * PAD
NBLK = 512             # PSUM bank width in f32
NSLOT = 6              # software pipeline depth (SBUF tiles)
NSLOT_PS = 4           # PSUM pipeline depth (2 banks per slot, 8 banks total)


def _band_stack():
    """bands[v, s, k, m] = BETA**(s + |di|), di = k - m - (0 if v==0 else R).

    v=0: top chunk (no upper halo), v=1: interior/bottom chunks.
    s in {0,1,2} is the |dj| scale baked into the stationary operand.
    """
    out = np.zeros((2, 3, 128, CO), dtype=np.float64)
    k = np.arange(128)[:, None]
    m = np.arange(CO)[None, :]
    for v, off in ((0, 0), (1, R)):
        d = np.abs(k - m - off)
        base = np.where(d <= R, float(BETA) ** d, 0.0)
        for s in range(3):
            out[v, s] = base * (float(BETA) ** s)
    return np.ascontiguousarray(out.astype(ml_dtypes.bfloat16))


def _const_inputs():
    return {"bands": _band_stack()}


def _build(nc, n_img, loop_niter=None):
    """loop_niter: wrap the body in a device-side For_i (timing only)."""
    x = nc.dram_tensor("x", [n_img, H, W], mybir.dt.float32, kind="ExternalInput")
    bands = nc.dram_tensor("bands", [2, 3, 128, CO], mybir.dt.bfloat16,
                           kind="ExternalInput")
    y = nc.dram_tensor("y", [n_img, H, W], mybir.dt.float32, kind="ExternalOutput")
    n_chunks = (H + CO - 1) // CO

    with tile.TileContext(nc) as tc, ExitStack() as ctx:
        wpool = ctx.enter_context(tc.tile_pool(name="weights", bufs=1))
        vpool = ctx.enter_context(tc.tile_pool(name="v", bufs=1))
        opool = ctx.enter_context(tc.tile_pool(name="out", bufs=1))
        ppool = ctx.enter_context(tc.tile_pool(name="psum", bufs=1, space="PSUM"))

        band_t = [[wpool.tile([128, CO], mybir.dt.bfloat16, name=f"band_{v}_{s}")
                   for s in range(3)] for v in range(2)]
        for v in range(2):
            for s in range(3):
                nc.sync.dma_start(band_t[v][s][:], bands[v, s])

        v0s = [vpool.tile([128, WP], mybir.dt.bfloat16, name=f"v0_{i}")
               for i in range(NSLOT)]
        ats = [vpool.tile([128, WP], mybir.dt.bfloat16, name=f"a_{i}")
               for i in range(NSLOT)]
        tss = [vpool.tile([128, W], mybir.dt.bfloat16, name=f"ts_{i}")
               for i in range(NSLOT)]
        v1s = [vpool.tile([128, WP], mybir.dt.bfloat16, name=f"v1_{i}")
               for i in range(NSLOT)]
        ots = [opool.tile([128, W], mybir.dt.float32, name=f"ot_{i}")
               for i in range(NSLOT)]
        pss = [ppool.tile([128, 2 * NBLK], mybir.dt.float32, name=f"ps_{i}")
               for i in range(NSLOT_PS)]
        mts = [vpool.tile([128, W], mybir.dt.float32, name=f"mt_{i}")
               for i in range(NSLOT)]
        sgs = [vpool.tile([128, W], mybir.dt.bfloat16, name=f"sg_{i}")
               for i in range(NSLOT)]
        ntau_t = wpool.tile([128, 1], mybir.dt.float32, name="ntau_t")
        nc.vector.memset(ntau_t[:], -float(TAU))
        half_t = wpool.tile([128, 1], mybir.dt.float32, name="half_t")
        nc.vector.memset(half_t[:], 0.5)
        for t in (*v0s, *v1s):
            nc.vector.memset(t[:, :PAD], 0.0)
            nc.vector.memset(t[:, PAD + W:], 0.0)

        loop = tc.For_i(0, loop_niter, 1) if loop_niter else None
        if loop:
            loop.__enter__()
        cidx = 0
        for img in range(n_img):
            for c in range(n_chunks):
                o0 = c * CO
                o1 = min(o0 + CO, H)
                i0 = max(o0 - R, 0)
                i1 = min(o1 + R, H)
                K = i1 - i0
                M = o1 - o0
                s = cidx % NSLOT
                ps = pss[cidx % NSLOT_PS]
                cidx += 1
                v0, a, ts, v1, ot = v0s[s], ats[s], tss[s], v1s[s], ots[s]
                mt, sg = mts[s], sgs[s]

                in_eng = nc.sync if (cidx % 2) else nc.scalar
                in_eng.dma_start(mt[:K, :], x[img, i0:i1, :])
                nc.vector.tensor_copy(v0[:K, PAD:PAD + W], mt[:K, :])

                nc.vector.tensor_tensor(
                    out=a[:K, PAD:PAD + W + 2],
                    in0=v0[:K, PAD - 4:PAD - 4 + W + 2],
                    in1=v0[:K, PAD + 2:PAD + 2 + W + 2],
                    op=mybir.AluOpType.max)
                nc.vector.tensor_scalar(
                    out=ts[:K, :], in0=a[:K, PAD + 1:PAD + 1 + W],
                    scalar1=float(B3), scalar2=None,
                    op0=mybir.AluOpType.mult)
                nc.vector.tensor_tensor(
                    out=v1[:K, PAD:PAD + W],
                    in0=ts[:K, :], in1=v0[:K, PAD:PAD + W],
                    op=mybir.AluOpType.max)

                bv = band_t[0 if c == 0 else 1]
                nmm = [0, 0]
                for s_idx, djs in ((2, (-2, 2)), (1, (-1, 1)), (0, (0,))):
                    lhsT = bv[s_idx][:K, :M]
                    for dj in djs:
                        for h in (0, 1):
                            col = PAD + h * NBLK + dj
                            nc.tensor.matmul(
                                ps[:M, h * NBLK:(h + 1) * NBLK],
                                lhsT, v1[:K, col:col + NBLK],
                                start=(nmm[h] == 0), stop=(nmm[h] == 4))
                            nmm[h] += 1

                nc.scalar.activation(sg[:M, :], ps[:M, :],
                                     mybir.ActivationFunctionType.Sign,
                                     bias=ntau_t[:M])
                nc.scalar.activation(ot[:M, :], sg[:M, :],
                                     mybir.ActivationFunctionType.Identity,
                                     bias=half_t[:M], scale=0.5)
                nc.gpsimd.dma_start(y[img, o0:o1, :], ot[:M, :])
        if loop:
            loop.__exit__(None, None, None)
    return nc


_CACHE = {}


def _get_nc():
    if "nc" not in _CACHE:
        nc = bacc.Bacc("TRN2", target_bir_lowering=False)
        _build(nc, PER_CORE)
        nc.compile()
        _CACHE["nc"] = nc
        _CACHE["consts"] = _const_inputs()
    return _CACHE["nc"], _CACHE["consts"]


def kernel(batch_mask, weight=None, iter_num=None, **_unused):
    from concourse.bass_utils import run_bass_kernel_spmd

    nc, consts = _get_nc()
    bm4 = np.ascontiguousarray(np.asarray(batch_mask, dtype=np.float32))
    assert bm4.shape == (B, C, H, W), bm4.shape
    in_maps = []
    for cidx in range(N_CORES):
        xs = np.ascontiguousarray(bm4[cidx * PER_CORE:(cidx + 1) * PER_CORE, 0])
        in_maps.append({"x": xs, **consts})
    res = run_bass_kernel_spmd(nc, in_maps, list(range(N_CORES)))
    out = np.concatenate([np.asarray(res.results[cidx]["y"])
                          for cidx in range(N_CORES)], axis=0)
    return out.reshape(B, C, H, W).astype(np.float32)
